# revision 6
# baseline (speedup 1.0000x reference)
"""Trainium2 Bass kernel for nn_AFF_1116691497756 (dense_cnn, AFF-style fusion).

Strategy: pure data parallelism over the batch dim (32 -> 4 per core, 8 cores).
All conv/BN params are folded on the host into effective weights/biases.
Heavy tensors travel and compute in bf16 (output converted back to f32 on
host); matmuls run on the PE in bf16; ReLU/sigmoid run on ACT fused with the
per-channel biases straight out of PSUM; the elementwise blends run on DVE in
bf16 2x mode. The global-pool branches are per-sample [C] vectors computed
from DVE accumulators feeding tiny N=1 matmuls.

Math per sample (all on device):
  Ah=0.5*x_a, Bh=0.5*x_b, Ch=0.5*x_c  (bf16 rows, with free-dim sums)
  h1   = relu(W1e0 @ (x_a+x_b) + B1e0)         [PE accumulates 2*W1e0@(Ah+Bh)]
  z1   = W2e0 @ h1;  bias1 = B2e0 + att_pool1  [pooled branch via sums]
  wei  = sigmoid(z1 + bias1); g1 = sigmoid(-z1 - bias1) = 1-wei
  XOh  = Bh + (Ah-Bh)*wei  = 0.5*xo1
  V    = Ch*g1 = 0.5*x_c*(1-wei)
  h2   = relu(W1e2 @ (xo1+x_c) + B1e2)         [PE accumulates 2*W1e2@(XOh+Ch)]
  z2   = W2e2 @ h2; wei2 = sigmoid(z2 + bias2)
  out  = XOh*(1+wei2) + V  = 0.5*(xo1 + xo2)
"""

import numpy as np
import ml_dtypes

import concourse.bass as bass
import concourse.bacc as bacc
import concourse.mybir as mybir
import concourse.tile as tile
from concourse.bass_utils import run_bass_kernel_spmd

EPS = 1e-5
N_CORES = 8

BF16 = mybir.dt.bfloat16
F32 = mybir.dt.float32
AOP = mybir.AluOpType
AF = mybir.ActivationFunctionType


class Cfg:
    def __init__(self, B=32, C=256, L=4096, I=64, Lc=512):
        self.B, self.C, self.L, self.I, self.Lc = B, C, L, I, Lc
        self.BL = B // N_CORES          # samples per core
        self.CH = C // 128              # C partition halves (2)
        self.NLC = L // Lc              # L chunks (8)
        assert C % 128 == 0 and L % Lc == 0 and self.NLC % 2 == 0


def build(cfg: Cfg):
    """Build the per-core SPMD program. Returns compiled Bacc."""
    BL, CH, L, I, Lc, NLC = cfg.BL, cfg.CH, cfg.L, cfg.I, cfg.Lc, cfg.NLC
    C = cfg.C

    nc = bacc.Bacc("TRN2", target_bir_lowering=False, debug=False,
                   num_devices=N_CORES)

    # ---- DRAM parameters ----
    xa = nc.declare_dram_parameter("xa", [BL, C, L], BF16, isOutput=False)
    xb = nc.declare_dram_parameter("xb", [BL, C, L], BF16, isOutput=False)
    xc = nc.declare_dram_parameter("xc", [BL, C, L], BF16, isOutput=False)
    # matmul weights (pre-transposed, scaled on host)
    lt1 = nc.declare_dram_parameter("lt1", [128, CH, I], BF16, isOutput=False)
    lt3 = nc.declare_dram_parameter("lt3", [128, CH, I], BF16, isOutput=False)
    lt2 = nc.declare_dram_parameter("lt2", [128, CH, 128], BF16, isOutput=False)
    lt4 = nc.declare_dram_parameter("lt4", [128, CH, 128], BF16, isOutput=False)
    ltp1 = nc.declare_dram_parameter("ltp1", [128, CH, I], BF16, isOutput=False)
    ltp3 = nc.declare_dram_parameter("ltp3", [128, CH, I], BF16, isOutput=False)
    ltp2 = nc.declare_dram_parameter("ltp2", [I, CH, 128], BF16, isOutput=False)
    ltp4 = nc.declare_dram_parameter("ltp4", [I, CH, 128], BF16, isOutput=False)
    # biases (f32)
    br1 = nc.declare_dram_parameter("br1", [128, 1], F32, isOutput=False)
    br2 = nc.declare_dram_parameter("br2", [128, 1], F32, isOutput=False)
    bp1 = nc.declare_dram_parameter("bp1", [I, 1], F32, isOutput=False)
    bp3 = nc.declare_dram_parameter("bp3", [I, 1], F32, isOutput=False)
    BB1 = nc.declare_dram_parameter("BB1", [128, CH], F32, isOutput=False)
    BB2 = nc.declare_dram_parameter("BB2", [128, CH], F32, isOutput=False)
    out = nc.declare_dram_parameter("out", [BL, C, L], BF16, isOutput=True)

    with tile.TileContext(nc) as tc:
        with (
            tc.tile_pool(name="const", bufs=1) as cpool,
            tc.tile_pool(name="rows_ab", bufs=6) as abpool,     # Ah,Bh rows
            tc.tile_pool(name="rows_c", bufs=3) as cpool_rows,  # Ch rows
            tc.tile_pool(name="rows_x", bufs=3) as xpool,       # XOh rows
            tc.tile_pool(name="rows_v", bufs=3) as vpool,       # V rows
            tc.tile_pool(name="rows_sig", bufs=4) as sigpool,   # w1s/g1/w2s/g2
            tc.tile_pool(name="rows_tmp", bufs=2) as tmppool,   # D/m/n
            tc.tile_pool(name="rows_out", bufs=2) as outpool,
            tc.tile_pool(name="junk", bufs=1) as junkpool,
            tc.tile_pool(name="hsb", bufs=2) as hpool,          # h1/h2 sbuf
            tc.tile_pool(name="small", bufs=2 * BL) as smallpool,
            tc.tile_pool(name="ppsum", bufs=2, space="PSUM") as ph_pool,
            tc.tile_pool(name="zpsum", bufs=4, space="PSUM") as pz_pool,
            tc.tile_pool(name="popsum", bufs=1, space="PSUM") as pp_pool,
        ):
            # ---- load constants to SBUF ----
            def cload(ap, shape, dtype, nm):
                t = cpool.tile(shape, dtype, name=nm, tag=nm)
                nc.sync.dma_start(t[:], ap[:])
                return t

            c_lt1 = cload(lt1, [128, CH, I], BF16, "c_lt1")
            c_lt3 = cload(lt3, [128, CH, I], BF16, "c_lt3")
            c_lt2 = cload(lt2, [128, CH, 128], BF16, "c_lt2")
            c_lt4 = cload(lt4, [128, CH, 128], BF16, "c_lt4")
            c_ltp1 = cload(ltp1, [128, CH, I], BF16, "c_ltp1")
            c_ltp3 = cload(ltp3, [128, CH, I], BF16, "c_ltp3")
            c_ltp2 = cload(ltp2, [I, CH, 128], BF16, "c_ltp2")
            c_ltp4 = cload(ltp4, [I, CH, 128], BF16, "c_ltp4")
            c_br1 = cload(br1, [128, 1], F32, "c_br1")
            c_br2 = cload(br2, [128, 1], F32, "c_br2")
            c_bp1 = cload(bp1, [I, 1], F32, "c_bp1")
            c_bp3 = cload(bp3, [I, 1], F32, "c_bp3")
            c_BB1 = cload(BB1, [128, CH], F32, "c_BB1")
            c_BB2 = cload(BB2, [128, CH], F32, "c_BB2")

            junk = junkpool.tile([128, L], BF16)

            def pooled_branch(sums_t, s0_col, s1_col, c_ltpA, c_bpA, c_ltpB,
                              c_BBx, mean_bf, bias_t, neg_t):
                """sums cols [s0..s0+CH) + [s1..s1+CH) -> bias vec [128, CH].
                Returns nothing; fills bias_t (and neg_t if not None)."""
                nc.vector.tensor_tensor(
                    mean_bf[:, :],
                    sums_t[:, s0_col:s0_col + CH],
                    sums_t[:, s1_col:s1_col + CH],
                    AOP.add)
                php = pp_pool.tile([I, 1], F32, tag="pp_h")
                for kh in range(CH):
                    nc.tensor.matmul(php[:, :], c_ltpA[:, kh, :],
                                     mean_bf[:, kh:kh + 1],
                                     start=(kh == 0), stop=(kh == CH - 1))
                hp = smallpool.tile([I, 1], BF16, tag="hp")
                nc.scalar.activation(hp[:, :], php[:, :], AF.Relu,
                                     bias=c_bpA[:, 0:1], scale=1.0)
                pat = pp_pool.tile([128, CH], F32, tag="pp_att")
                for mh in range(CH):
                    nc.tensor.matmul(pat[:, mh:mh + 1], c_ltpB[:, mh, :],
                                     hp[:, 0:1], start=True, stop=True)
                # bias = att_psum + BBx  (per C-half columns)
                for mh in range(CH):
                    nc.vector.tensor_scalar(
                        bias_t[:, mh:mh + 1], pat[:, mh:mh + 1],
                        c_BBx[:, mh:mh + 1], None, AOP.add)
                if neg_t is not None:
                    nc.vector.tensor_scalar(
                        neg_t[:, :], bias_t[:, :], -1.0, None, AOP.mult)

            for b in range(BL):
                # ---------------- phase A: load + halve + sums ----------
                # sums layout: [sA(CH), sB(CH), sC(CH), sX(CH)]
                sums = smallpool.tile([128, 4 * CH], F32, tag="sums")
                tA, tB, tC = [], [], []
                for kh in range(CH):
                    ta = abpool.tile([128, L], BF16, tag="ab")
                    nc.sync.dma_start(ta[:], xa[b, kh * 128:(kh + 1) * 128, :])
                    nc.vector.tensor_scalar(
                        ta[:], ta[:], 0.5, None, AOP.mult, AOP.add,
                        accum_out=sums[:, 0 * CH + kh:0 * CH + kh + 1])
                    tA.append(ta)
                    tb = abpool.tile([128, L], BF16, tag="ab")
                    nc.sync.dma_start(tb[:], xb[b, kh * 128:(kh + 1) * 128, :])
                    nc.vector.tensor_scalar(
                        tb[:], tb[:], 0.5, None, AOP.mult, AOP.add,
                        accum_out=sums[:, 1 * CH + kh:1 * CH + kh + 1])
                    tB.append(tb)
                    tcc = cpool_rows.tile([128, L], BF16, tag="c")
                    nc.sync.dma_start(tcc[:], xc[b, kh * 128:(kh + 1) * 128, :])
                    nc.vector.tensor_scalar(
                        tcc[:], tcc[:], 0.5, None, AOP.mult, AOP.add,
                        accum_out=sums[:, 2 * CH + kh:2 * CH + kh + 1])
                    tC.append(tcc)

                # ---------------- pooled branch 1 ----------------------
                mean1 = smallpool.tile([128, CH], BF16, tag="mean")
                bias1 = smallpool.tile([128, CH], F32, tag="bias1")
                nb1 = smallpool.tile([128, CH], F32, tag="nb1")
                pooled_branch(sums, 0 * CH, 1 * CH, c_ltp1, c_bp1, c_ltp2,
                              c_BB1, mean1, bias1, nb1)

                # ---------------- phase B ------------------------------
                w1s = [sigpool.tile([128, L], BF16, tag="sig",
                                    name=f"w1s_{b}_{i}") for i in range(CH)]
                g1 = [sigpool.tile([128, L], BF16, tag="sig",
                                   name=f"g1_{b}_{i}") for i in range(CH)]
                for lcp in range(NLC // 2):
                    ph = ph_pool.tile([128, Lc], F32, tag="ph")
                    for sub in range(2):
                        lc = 2 * lcp + sub
                        sl = slice(lc * Lc, (lc + 1) * Lc)
                        n_mm = 2 * CH
                        i_mm = 0
                        for t_in in (tA, tB):
                            for kh in range(CH):
                                nc.tensor.matmul(
                                    ph[64 * sub:64 * sub + I, :],
                                    c_lt1[:, kh, :], t_in[kh][:, sl],
                                    start=(i_mm == 0), stop=(i_mm == n_mm - 1))
                                i_mm += 1
                    h1 = hpool.tile([128, Lc], BF16, tag="h")
                    nc.scalar.activation(h1[:], ph[:], AF.Relu,
                                         bias=c_br1[:, 0:1], scale=1.0)
                    for sub in range(2):
                        lc = 2 * lcp + sub
                        sl = slice(lc * Lc, (lc + 1) * Lc)
                        for mh in range(CH):
                            pz = pz_pool.tile([128, Lc], F32, tag="pz")
                            nc.tensor.matmul(
                                pz[:, :],
                                c_lt2[64 * sub:64 * sub + I, mh, :],
                                h1[64 * sub:64 * sub + I, :],
                                start=True, stop=True)
                            nc.scalar.activation(
                                w1s[mh][:, sl], pz[:], AF.Sigmoid,
                                bias=bias1[:, mh:mh + 1], scale=1.0)
                            nc.scalar.activation(
                                g1[mh][:, sl], pz[:], AF.Sigmoid,
                                bias=nb1[:, mh:mh + 1], scale=-1.0)

                # full-row DVE: XOh, V, sums of XOh
                tX, tV = [], []
                for mh in range(CH):
                    D = tmppool.tile([128, L], BF16, tag="tmp")
                    nc.vector.tensor_tensor(D[:], tA[mh][:], tB[mh][:],
                                            AOP.subtract)
                    m = tmppool.tile([128, L], BF16, tag="tmp")
                    nc.vector.tensor_tensor(m[:], D[:], w1s[mh][:], AOP.mult)
                    x_t = xpool.tile([128, L], BF16, tag="x")
                    nc.vector.tensor_tensor(x_t[:], tB[mh][:], m[:], AOP.add)
                    tX.append(x_t)
                    v_t = vpool.tile([128, L], BF16, tag="v")
                    nc.vector.tensor_tensor(v_t[:], tC[mh][:], g1[mh][:],
                                            AOP.mult)
                    tV.append(v_t)
                    nc.vector.tensor_scalar(
                        junk[:], x_t[:], 1.0, None, AOP.mult, AOP.add,
                        accum_out=sums[:, 3 * CH + mh:3 * CH + mh + 1])

                # ---------------- pooled branch 2 ----------------------
                mean2 = smallpool.tile([128, CH], BF16, tag="mean2")
                bias2 = smallpool.tile([128, CH], F32, tag="bias2")
                pooled_branch(sums, 3 * CH, 2 * CH, c_ltp3, c_bp3, c_ltp4,
                              c_BB2, mean2, bias2, None)

                # ---------------- phase C ------------------------------
                w2s = [sigpool.tile([128, L], BF16, tag="sig",
                                    name=f"w2s_{b}_{i}") for i in range(CH)]
                for lcp in range(NLC // 2):
                    ph2 = ph_pool.tile([128, Lc], F32, tag="ph")
                    for sub in range(2):
                        lc = 2 * lcp + sub
                        sl = slice(lc * Lc, (lc + 1) * Lc)
                        n_mm = 2 * CH
                        i_mm = 0
                        for t_in in (tX, tC):
                            for kh in range(CH):
                                nc.tensor.matmul(
                                    ph2[64 * sub:64 * sub + I, :],
                                    c_lt3[:, kh, :], t_in[kh][:, sl],
                                    start=(i_mm == 0), stop=(i_mm == n_mm - 1))
                                i_mm += 1
                    h2 = hpool.tile([128, Lc], BF16, tag="h")
                    nc.scalar.activation(h2[:], ph2[:], AF.Relu,
                                         bias=c_br2[:, 0:1], scale=1.0)
                    for sub in range(2):
                        lc = 2 * lcp + sub
                        sl = slice(lc * Lc, (lc + 1) * Lc)
                        for mh in range(CH):
                            pz2 = pz_pool.tile([128, Lc], F32, tag="pz")
                            nc.tensor.matmul(
                                pz2[:, :],
                                c_lt4[64 * sub:64 * sub + I, mh, :],
                                h2[64 * sub:64 * sub + I, :],
                                start=True, stop=True)
                            nc.scalar.activation(
                                w2s[mh][:, sl], pz2[:], AF.Sigmoid,
                                bias=bias2[:, mh:mh + 1], scale=1.0)

                for mh in range(CH):
                    g2 = sigpool.tile([128, L], BF16, tag="sig")
                    nc.vector.tensor_scalar(g2[:], w2s[mh][:], 1.0, None,
                                            AOP.add)
                    n_t = tmppool.tile([128, L], BF16, tag="tmp")
                    nc.vector.tensor_tensor(n_t[:], tX[mh][:], g2[:], AOP.mult)
                    ob = outpool.tile([128, L], BF16, tag="ob")
                    nc.vector.tensor_tensor(ob[:], n_t[:], tV[mh][:], AOP.add)
                    nc.sync.dma_start(out[b, mh * 128:(mh + 1) * 128, :],
                                      ob[:])

    nc.compile()
    return nc


def host_params(w1, b1, bn1_g, bn1_b, bn1_m, bn1_v,
                w2, b2, bn2_g, bn2_b, bn2_m, bn2_v, cfg: Cfg):
    """Fold BN into conv weights; build device param arrays."""
    CH, I, L = cfg.CH, cfg.I, cfg.L
    w1 = w1.astype(np.float64); w2 = w2.astype(np.float64)
    s1 = bn1_g / np.sqrt(bn1_v + EPS)            # [4, I]
    t1 = bn1_b - bn1_m * s1
    W1e = s1[:, :, None] * w1                    # [4, I, C]
    B1e = s1 * b1 + t1                           # [4, I]
    s2 = bn2_g / np.sqrt(bn2_v + EPS)            # [4, C]
    t2 = bn2_b - bn2_m * s2
    W2e = s2[:, :, None] * w2                    # [4, C, I]
    B2e = s2 * b2 + t2                           # [4, C]

    def to_bf(x):
        return np.ascontiguousarray(x.astype(ml_dtypes.bfloat16))

    def kxm(W, scale):  # W [I, C] -> lhsT [128, CH, I]
        t = (W.T * scale).reshape(CH, 128, I).transpose(1, 0, 2)
        return to_bf(t)

    def dup_mt(W):  # W [C, I] -> duplicated lhsT [128, CH, 128]
        t = W.T.reshape(I, CH, 128)              # [I, CH, 128]
        return to_bf(np.concatenate([t, t], axis=0))  # [2I=128, CH, 128]

    def mt(W):  # W [C, I] -> lhsT [I, CH, 128]
        return to_bf(W.T.reshape(I, CH, 128))

    p = {
        "lt1": kxm(W1e[0], 2.0),
        "lt3": kxm(W1e[2], 2.0),
        "lt2": dup_mt(W2e[0]),
        "lt4": dup_mt(W2e[2]),
        "ltp1": kxm(W1e[1], 2.0 / L),
        "ltp3": kxm(W1e[3], 2.0 / L),
        "ltp2": mt(W2e[1]),
        "ltp4": mt(W2e[3]),
        "br1": np.concatenate([B1e[0], B1e[0]]).astype(np.float32)
                 .reshape(128, 1),
        "br2": np.concatenate([B1e[2], B1e[2]]).astype(np.float32)
                 .reshape(128, 1),
        "bp1": B1e[1].astype(np.float32).reshape(I, 1),
        "bp3": B1e[3].astype(np.float32).reshape(I, 1),
        "BB1": (B2e[0] + B2e[1]).astype(np.float32).reshape(CH, 128).T.copy(),
        "BB2": (B2e[2] + B2e[3]).astype(np.float32).reshape(CH, 128).T.copy(),
    }
    return p


_CACHE = {}


def _get_nc(cfg: Cfg):
    key = (cfg.B, cfg.C, cfg.L, cfg.I, cfg.Lc)
    if key not in _CACHE:
        _CACHE[key] = build(cfg)
    return _CACHE[key]


LAST_RESULT = [None]


def kernel(x_a, x_b, x_c, w1, b1, bn1_g, bn1_b, bn1_m, bn1_v,
           w2, b2, bn2_g, bn2_b, bn2_m, bn2_v):
    cfg = Cfg(B=x_a.shape[0], C=x_a.shape[1], L=x_a.shape[2], I=w1.shape[1])
    nc = _get_nc(cfg)
    params = host_params(np.asarray(w1), np.asarray(b1), np.asarray(bn1_g),
                         np.asarray(bn1_b), np.asarray(bn1_m),
                         np.asarray(bn1_v), np.asarray(w2), np.asarray(b2),
                         np.asarray(bn2_g), np.asarray(bn2_b),
                         np.asarray(bn2_m), np.asarray(bn2_v), cfg)
    BL = cfg.BL
    bf = ml_dtypes.bfloat16
    in_maps = []
    for i in range(N_CORES):
        sl = slice(i * BL, (i + 1) * BL)
        m = dict(params)
        m["xa"] = np.ascontiguousarray(np.asarray(x_a[sl]).astype(bf))
        m["xb"] = np.ascontiguousarray(np.asarray(x_b[sl]).astype(bf))
        m["xc"] = np.ascontiguousarray(np.asarray(x_c[sl]).astype(bf))
        in_maps.append(m)

    import os
    res = run_bass_kernel_spmd(nc, in_maps, core_ids=list(range(N_CORES)),
                               trace=bool(os.environ.get("BASS_TRACE")))
    LAST_RESULT[0] = res
    out = np.concatenate([res.results[i]["out"].astype(np.float32)
                          for i in range(N_CORES)], axis=0)
    return out


# revision 9
# speedup vs baseline: 1.0491x; 1.0491x over previous
"""Trainium2 Bass kernel for nn_AFF_1116691497756 (dense_cnn, AFF-style fusion).

Pure data parallelism over batch (32 -> 4 per core, 8 cores). BN folded into
conv weights on host. Inputs ship as bf16 pre-halved (0.5*x, exact scaling);
output returns bf16 and is widened on host.

Key structure per core sample:
  S  = Ah+Bh, D = Ah-Bh                          [DVE TT bf16 2x]
  mm1: psum[0:64]  = 2*W1e0 @ S   (h1 pre-act)   [PE, K=256]
       psum[64:128]= (2/L)*W1e1 @ S (pooled rows, summed over L via ACT accum)
  h1 = relu(psum[0:64] + B1e0)                   [ACT from PSUM]
  pool1: relu(sum rows + B1e1) -> W2e1 -> bias1  [tiny]
  z1 = W2e0 @ h1                                 [PE K=64]
  T1 = tanh(0.5*(z1+bias1)) (= 2*wei-1)          [ACT from PSUM]
  g1 = sigmoid(-(z1+bias1)) (= 1-wei)            [ACT from PSUM]
  XQ = S + D*T1 (= xo1);  V = Ch*g1              [DVE TT]
  mm3: psum[0:64]  = W1e2@XQ + 2*W1e2@Ch  (= W1e2@(xo1+x_c))
       psum[64:128]= (1/L)*W1e3@XQ + (2/L)*W1e3@Ch (pooled rows)
  h2 = relu(... + B1e2); pool2 -> bias2; z2 = W2e2@h2
  w2s = sigmoid(z2+bias2); g2 = 0.5 + 0.5*w2s    [ACT; DVE TS 4x]
  out = XQ*g2 + V  (= 0.5*(xo1+xo2))             [DVE TT x2]
"""

import numpy as np
import ml_dtypes

import concourse.bass as bass
import concourse.bacc as bacc
import concourse.mybir as mybir
import concourse.tile as tile
from concourse.bass_utils import run_bass_kernel_spmd

EPS = 1e-5
N_CORES = 8

BF16 = mybir.dt.bfloat16
F32 = mybir.dt.float32
AOP = mybir.AluOpType
AF = mybir.ActivationFunctionType


class Cfg:
    def __init__(self, B=32, C=256, L=4096, I=64, Lc=512):
        self.B, self.C, self.L, self.I, self.Lc = B, C, L, I, Lc
        self.BL = B // N_CORES          # samples per core
        self.CH = C // 128              # C partition halves (2)
        self.NLC = L // Lc              # L chunks (8)
        assert C % 128 == 0 and L % Lc == 0 and self.NLC % 2 == 0
        assert I == 64 and self.CH == 2


def build(cfg: Cfg):
    """Build the per-core SPMD program. Returns compiled Bacc."""
    BL, CH, L, I, Lc, NLC = cfg.BL, cfg.CH, cfg.L, cfg.I, cfg.Lc, cfg.NLC
    C = cfg.C
    NG = NLC // 2                       # psum groups (2 chunks each)

    nc = bacc.Bacc("TRN2", target_bir_lowering=False, debug=False,
                   num_devices=N_CORES)

    # ---- DRAM parameters ----
    xa = nc.declare_dram_parameter("xa", [BL, C, L], BF16, isOutput=False)
    xb = nc.declare_dram_parameter("xb", [BL, C, L], BF16, isOutput=False)
    xc = nc.declare_dram_parameter("xc", [BL, C, L], BF16, isOutput=False)
    # mm1/mm3 weights: [K=128, CH, M=128] with pooled weights in cols 64:128
    lt1 = nc.declare_dram_parameter("lt1", [128, CH, 128], BF16,
                                    isOutput=False)
    lt3a = nc.declare_dram_parameter("lt3a", [128, CH, 128], BF16,
                                     isOutput=False)
    lt3b = nc.declare_dram_parameter("lt3b", [128, CH, 128], BF16,
                                     isOutput=False)
    # mm2/mm4 weights: [K=64, CH, 128]
    lt2 = nc.declare_dram_parameter("lt2", [I, CH, 128], BF16, isOutput=False)
    lt4 = nc.declare_dram_parameter("lt4", [I, CH, 128], BF16, isOutput=False)
    # pooled second-layer weights, rows 64:128 hold W2p^T (for base-64 rhs)
    ltp2 = nc.declare_dram_parameter("ltp2", [128, CH, 128], BF16,
                                     isOutput=False)
    ltp4 = nc.declare_dram_parameter("ltp4", [128, CH, 128], BF16,
                                     isOutput=False)
    br1 = nc.declare_dram_parameter("br1", [128, 1], F32, isOutput=False)
    br2 = nc.declare_dram_parameter("br2", [128, 1], F32, isOutput=False)
    bp1 = nc.declare_dram_parameter("bp1", [128, 1], F32, isOutput=False)
    bp3 = nc.declare_dram_parameter("bp3", [128, 1], F32, isOutput=False)
    BB1 = nc.declare_dram_parameter("BB1", [128, CH], F32, isOutput=False)
    BB2 = nc.declare_dram_parameter("BB2", [128, CH], F32, isOutput=False)
    out = nc.declare_dram_parameter("out", [BL, C, L], BF16, isOutput=True)

    with tile.TileContext(nc) as tc:
        with (
            tc.tile_pool(name="const", bufs=1) as cpool,
            tc.tile_pool(name="rows_ab", bufs=4) as abpool,     # Ah,Bh rows
            tc.tile_pool(name="rows_c", bufs=3) as cpool_rows,  # Ch rows
            tc.tile_pool(name="rows_s", bufs=3) as spool,       # S rows
            tc.tile_pool(name="rows_x", bufs=2) as xpool,       # XQ rows
            tc.tile_pool(name="rows_v", bufs=2) as vpool,       # V rows
            tc.tile_pool(name="rows_sig", bufs=4) as sigpool,   # T1/g1/w2s
            tc.tile_pool(name="rows_tmp", bufs=3) as tmppool,   # D/m/n/g2
            tc.tile_pool(name="rows_out", bufs=2) as outpool,
            tc.tile_pool(name="junk", bufs=1) as junkpool,
            tc.tile_pool(name="hsb", bufs=7) as hpool,          # h1/h2 sbuf
            tc.tile_pool(name="small", bufs=2 * BL) as smallpool,
            tc.tile_pool(name="hpsum", bufs=2, space="PSUM") as ph_pool,
            tc.tile_pool(name="zpsum", bufs=2, space="PSUM") as pz_pool,
        ):
            # ---- load constants to SBUF ----
            def cload(ap, shape, dtype, nm):
                t = cpool.tile(shape, dtype, name=nm, tag=nm)
                nc.sync.dma_start(t[:], ap[:])
                return t

            c_lt1 = cload(lt1, [128, CH, 128], BF16, "c_lt1")
            c_lt3a = cload(lt3a, [128, CH, 128], BF16, "c_lt3a")
            c_lt3b = cload(lt3b, [128, CH, 128], BF16, "c_lt3b")
            c_lt2 = cload(lt2, [I, CH, 128], BF16, "c_lt2")
            c_lt4 = cload(lt4, [I, CH, 128], BF16, "c_lt4")
            c_ltp2 = cload(ltp2, [128, CH, 128], BF16, "c_ltp2")
            c_ltp4 = cload(ltp4, [128, CH, 128], BF16, "c_ltp4")
            c_br1 = cload(br1, [128, 1], F32, "c_br1")
            c_br2 = cload(br2, [128, 1], F32, "c_br2")
            c_bp1 = cload(bp1, [128, 1], F32, "c_bp1")
            c_bp3 = cload(bp3, [128, 1], F32, "c_bp3")
            c_BB1 = cload(BB1, [128, CH], F32, "c_BB1")
            c_BB2 = cload(BB2, [128, CH], F32, "c_BB2")

            junk = junkpool.tile([128, 2 * Lc], BF16)

            def pooled_head(s4, cols, c_bpA, c_ltpB, nm):
                """Reduce pooled partial sums -> relu -> W2p -> att psum."""
                pre = smallpool.tile([128, 1], F32, tag="pre",
                                     name=f"pre{nm}")
                nc.vector.tensor_reduce(pre[64:128, 0:1], s4[64:128, cols],
                                        mybir.AxisListType.X, AOP.add)
                hp = smallpool.tile([128, 1], BF16, tag="hp", name=f"hp{nm}")
                nc.scalar.activation(hp[64:128, 0:1], pre[64:128, 0:1],
                                     AF.Relu, bias=c_bpA[64:128, 0:1],
                                     scale=1.0)
                pat = pz_pool.tile([128, CH], F32, tag="pz", name=f"pat{nm}")
                for mh in range(CH):
                    nc.tensor.matmul(pat[:, mh:mh + 1],
                                     c_ltpB[64:128, mh, :],
                                     hp[64:128, 0:1], start=True, stop=True)
                return pat

            for b in range(BL):
                # -------- phase A: load + S/D ---------------------------
                s4 = smallpool.tile([128, 2 * NG], F32, tag="s4",
                                    name=f"s4_{b}")
                tA, tB, tC, tS, tD = [], [], [], [], []
                for kh in range(CH):
                    ta = abpool.tile([128, L], BF16, tag="ab",
                                     name=f"ta_{b}_{kh}")
                    nc.sync.dma_start(ta[:], xa[b, kh * 128:(kh + 1) * 128, :])
                    tA.append(ta)
                    tb = abpool.tile([128, L], BF16, tag="ab",
                                     name=f"tb_{b}_{kh}")
                    nc.sync.dma_start(tb[:], xb[b, kh * 128:(kh + 1) * 128, :])
                    tB.append(tb)
                    tcc = cpool_rows.tile([128, L], BF16, tag="c",
                                          name=f"tc_{b}_{kh}")
                    nc.sync.dma_start(tcc[:],
                                      xc[b, kh * 128:(kh + 1) * 128, :])
                    tC.append(tcc)
                    ts_ = spool.tile([128, L], BF16, tag="s",
                                     name=f"ts_{b}_{kh}")
                    nc.vector.tensor_tensor(ts_[:], ta[:], tb[:], AOP.add)
                    tS.append(ts_)
                    td_ = tmppool.tile([128, L], BF16, tag="d",
                                       name=f"td_{b}_{kh}")
                    nc.vector.tensor_tensor(td_[:], ta[:], tb[:],
                                            AOP.subtract)
                    tD.append(td_)

                # -------- phase B1: mm1 (+pooled rows) + relu ----------
                h1s = []
                for g in range(NG):
                    ph = ph_pool.tile([128, 2 * Lc], F32, tag="ph",
                                      name=f"ph_{b}_{g}")
                    for q in range(2):
                        lc = 2 * g + q
                        sl = slice(lc * Lc, (lc + 1) * Lc)
                        for kh in range(CH):
                            nc.tensor.matmul(
                                ph[:, q * Lc:(q + 1) * Lc],
                                c_lt1[:, kh, :], tS[kh][:, sl],
                                start=(kh == 0), stop=(kh == CH - 1))
                    h1 = hpool.tile([I, 2 * Lc], BF16, tag="h",
                                    name=f"h1_{b}_{g}")
                    nc.scalar.activation(h1[:], ph[0:I, :], AF.Relu,
                                         bias=c_br1[0:I, 0:1], scale=1.0)
                    h1s.append(h1)
                    nc.scalar.activation(
                        junk[64:128, :], ph[64:128, :], AF.Copy, bias=0.0,
                        scale=1.0, accum_out=s4[64:128, g:g + 1])

                # -------- pooled branch 1 ------------------------------
                pat1 = pooled_head(s4, slice(0, NG), c_bp1, c_ltp2, f"1_{b}")
                bias1h = smallpool.tile([128, CH], F32, tag="bias1h",
                                        name=f"bias1h_{b}")
                nb1 = smallpool.tile([128, CH], F32, tag="nb1",
                                     name=f"nb1_{b}")
                for mh in range(CH):
                    nc.vector.tensor_scalar(
                        bias1h[:, mh:mh + 1], pat1[:, mh:mh + 1],
                        c_BB1[:, mh:mh + 1], 0.5, AOP.add, AOP.mult)
                    nc.vector.tensor_scalar(
                        nb1[:, mh:mh + 1], pat1[:, mh:mh + 1],
                        c_BB1[:, mh:mh + 1], -1.0, AOP.add, AOP.mult)

                # -------- phase B2: mm2 -> T1/g1 -----------------------
                T1 = [sigpool.tile([128, L], BF16, tag="sig",
                                   name=f"T1_{b}_{i}") for i in range(CH)]
                g1 = [sigpool.tile([128, L], BF16, tag="sig",
                                   name=f"g1_{b}_{i}") for i in range(CH)]
                for g in range(NG):
                    for mh in range(CH):
                        pz = pz_pool.tile([128, 2 * Lc], F32, tag="pz",
                                          name=f"pz_{b}_{g}_{mh}")
                        for q in range(2):
                            nc.tensor.matmul(
                                pz[:, q * Lc:(q + 1) * Lc],
                                c_lt2[:, mh, :],
                                h1s[g][:, q * Lc:(q + 1) * Lc],
                                start=True, stop=True)
                        sl2 = slice(2 * g * Lc, 2 * (g + 1) * Lc)
                        nc.scalar.activation(
                            T1[mh][:, sl2], pz[:], AF.Tanh,
                            bias=bias1h[:, mh:mh + 1], scale=0.5)
                        nc.scalar.activation(
                            g1[mh][:, sl2], pz[:], AF.Sigmoid,
                            bias=nb1[:, mh:mh + 1], scale=-1.0)

                # full-row DVE: XQ = S + D*T1 (= xo1), V = Ch*g1
                tX, tV = [], []
                for mh in range(CH):
                    m_t = tmppool.tile([128, L], BF16, tag="d",
                                       name=f"m_{b}_{mh}")
                    nc.vector.tensor_tensor(m_t[:], tD[mh][:], T1[mh][:],
                                            AOP.mult)
                    x_t = xpool.tile([128, L], BF16, tag="x",
                                     name=f"x_{b}_{mh}")
                    nc.vector.tensor_tensor(x_t[:], tS[mh][:], m_t[:],
                                            AOP.add)
                    tX.append(x_t)
                    v_t = vpool.tile([128, L], BF16, tag="v",
                                     name=f"v_{b}_{mh}")
                    nc.vector.tensor_tensor(v_t[:], tC[mh][:], g1[mh][:],
                                            AOP.mult)
                    tV.append(v_t)

                # -------- phase C1: mm3 (+pooled rows) + relu ----------
                h2s = []
                for g in range(NG):
                    ph2 = ph_pool.tile([128, 2 * Lc], F32, tag="ph",
                                       name=f"ph2_{b}_{g}")
                    for q in range(2):
                        lc = 2 * g + q
                        sl = slice(lc * Lc, (lc + 1) * Lc)
                        i_mm = 0
                        for t_in, lt_ in ((tX, c_lt3a), (tC, c_lt3b)):
                            for kh in range(CH):
                                nc.tensor.matmul(
                                    ph2[:, q * Lc:(q + 1) * Lc],
                                    lt_[:, kh, :], t_in[kh][:, sl],
                                    start=(i_mm == 0),
                                    stop=(i_mm == 2 * CH - 1))
                                i_mm += 1
                    h2 = hpool.tile([I, 2 * Lc], BF16, tag="h",
                                    name=f"h2_{b}_{g}")
                    nc.scalar.activation(h2[:], ph2[0:I, :], AF.Relu,
                                         bias=c_br2[0:I, 0:1], scale=1.0)
                    h2s.append(h2)
                    nc.scalar.activation(
                        junk[64:128, :], ph2[64:128, :], AF.Copy, bias=0.0,
                        scale=1.0, accum_out=s4[64:128, NG + g:NG + g + 1])

                # -------- pooled branch 2 ------------------------------
                pat2 = pooled_head(s4, slice(NG, 2 * NG), c_bp3, c_ltp4,
                                   f"2_{b}")
                bias2 = smallpool.tile([128, CH], F32, tag="bias2",
                                       name=f"bias2_{b}")
                for mh in range(CH):
                    nc.vector.tensor_scalar(
                        bias2[:, mh:mh + 1], pat2[:, mh:mh + 1],
                        c_BB2[:, mh:mh + 1], None, AOP.add)

                # -------- phase C2: mm4 -> w2s -------------------------
                w2s = [sigpool.tile([128, L], BF16, tag="sig",
                                    name=f"w2s_{b}_{i}") for i in range(CH)]
                for g in range(NG):
                    for mh in range(CH):
                        pz2 = pz_pool.tile([128, 2 * Lc], F32, tag="pz",
                                           name=f"pz2_{b}_{g}_{mh}")
                        for q in range(2):
                            nc.tensor.matmul(
                                pz2[:, q * Lc:(q + 1) * Lc],
                                c_lt4[:, mh, :],
                                h2s[g][:, q * Lc:(q + 1) * Lc],
                                start=True, stop=True)
                        sl2 = slice(2 * g * Lc, 2 * (g + 1) * Lc)
                        nc.scalar.activation(
                            w2s[mh][:, sl2], pz2[:], AF.Sigmoid,
                            bias=bias2[:, mh:mh + 1], scale=1.0)

                for mh in range(CH):
                    g2 = tmppool.tile([128, L], BF16, tag="d",
                                      name=f"g2_{b}_{mh}")
                    nc.vector.tensor_scalar(g2[:], w2s[mh][:], 0.5, 0.5,
                                            AOP.mult, AOP.add)
                    n_t = tmppool.tile([128, L], BF16, tag="d",
                                       name=f"n_{b}_{mh}")
                    nc.vector.tensor_tensor(n_t[:], tX[mh][:], g2[:],
                                            AOP.mult)
                    ob = outpool.tile([128, L], BF16, tag="ob",
                                      name=f"ob_{b}_{mh}")
                    nc.vector.tensor_tensor(ob[:], n_t[:], tV[mh][:], AOP.add)
                    nc.sync.dma_start(out[b, mh * 128:(mh + 1) * 128, :],
                                      ob[:])

    nc.compile()
    return nc


def host_params(w1, b1, bn1_g, bn1_b, bn1_m, bn1_v,
                w2, b2, bn2_g, bn2_b, bn2_m, bn2_v, cfg: Cfg):
    """Fold BN into conv weights; build device param arrays."""
    CH, I, L = cfg.CH, cfg.I, cfg.L
    w1 = w1.astype(np.float64); w2 = w2.astype(np.float64)
    s1 = bn1_g / np.sqrt(bn1_v + EPS)            # [4, I]
    t1 = bn1_b - bn1_m * s1
    W1e = s1[:, :, None] * w1                    # [4, I, C]
    B1e = s1 * b1 + t1                           # [4, I]
    s2 = bn2_g / np.sqrt(bn2_v + EPS)            # [4, C]
    t2 = bn2_b - bn2_m * s2
    W2e = s2[:, :, None] * w2                    # [4, C, I]
    B2e = s2 * b2 + t2                           # [4, C]

    def to_bf(x):
        return np.ascontiguousarray(x.astype(ml_dtypes.bfloat16))

    def kxm_ext(Wf, sf, Wp, sp):
        # [I,C] full (scale sf) + [I,C] pooled (scale sp)
        # -> lhsT [128, CH, 128]: cols 0:64 full, 64:128 pooled
        full = (Wf.T * sf).reshape(CH, 128, I)       # [CH, 128, I]
        pool = (Wp.T * sp).reshape(CH, 128, I)
        t = np.concatenate([full, pool], axis=2)      # [CH, 128, 128]
        return to_bf(t.transpose(1, 0, 2))            # [128, CH, 128]

    def mt(W):  # W [C, I] -> lhsT [I, CH, 128]
        return to_bf(W.T.reshape(I, CH, 128))

    def dup_mt(W):  # W [C, I] -> [128, CH, 128], rows 64:128 = W^T slices
        t = W.T.reshape(I, CH, 128)
        return to_bf(np.concatenate([t, t], axis=0))

    p = {
        "lt1": kxm_ext(W1e[0], 2.0, W1e[1], 2.0 / L),
        "lt3a": kxm_ext(W1e[2], 1.0, W1e[3], 1.0 / L),
        "lt3b": kxm_ext(W1e[2], 2.0, W1e[3], 2.0 / L),
        "lt2": mt(W2e[0]),
        "lt4": mt(W2e[2]),
        "ltp2": dup_mt(W2e[1]),
        "ltp4": dup_mt(W2e[3]),
        "br1": np.concatenate([B1e[0], B1e[0]]).astype(np.float32)
                 .reshape(128, 1),
        "br2": np.concatenate([B1e[2], B1e[2]]).astype(np.float32)
                 .reshape(128, 1),
        "bp1": np.concatenate([B1e[1], B1e[1]]).astype(np.float32)
                 .reshape(128, 1),
        "bp3": np.concatenate([B1e[3], B1e[3]]).astype(np.float32)
                 .reshape(128, 1),
        "BB1": (B2e[0] + B2e[1]).astype(np.float32).reshape(CH, 128).T.copy(),
        "BB2": (B2e[2] + B2e[3]).astype(np.float32).reshape(CH, 128).T.copy(),
    }
    return p


_CACHE = {}


def _get_nc(cfg: Cfg):
    key = (cfg.B, cfg.C, cfg.L, cfg.I, cfg.Lc)
    if key not in _CACHE:
        _CACHE[key] = build(cfg)
    return _CACHE[key]


LAST_RESULT = [None]


def kernel(x_a, x_b, x_c, w1, b1, bn1_g, bn1_b, bn1_m, bn1_v,
           w2, b2, bn2_g, bn2_b, bn2_m, bn2_v):
    cfg = Cfg(B=x_a.shape[0], C=x_a.shape[1], L=x_a.shape[2], I=w1.shape[1])
    nc = _get_nc(cfg)
    params = host_params(np.asarray(w1), np.asarray(b1), np.asarray(bn1_g),
                         np.asarray(bn1_b), np.asarray(bn1_m),
                         np.asarray(bn1_v), np.asarray(w2), np.asarray(b2),
                         np.asarray(bn2_g), np.asarray(bn2_b),
                         np.asarray(bn2_m), np.asarray(bn2_v), cfg)
    BL = cfg.BL
    bf = ml_dtypes.bfloat16
    in_maps = []
    for i in range(N_CORES):
        sl = slice(i * BL, (i + 1) * BL)
        m = dict(params)
        m["xa"] = np.ascontiguousarray((np.asarray(x_a[sl]) * 0.5).astype(bf))
        m["xb"] = np.ascontiguousarray((np.asarray(x_b[sl]) * 0.5).astype(bf))
        m["xc"] = np.ascontiguousarray((np.asarray(x_c[sl]) * 0.5).astype(bf))
        in_maps.append(m)

    import os
    res = run_bass_kernel_spmd(nc, in_maps, core_ids=list(range(N_CORES)),
                               trace=bool(os.environ.get("BASS_TRACE")))
    LAST_RESULT[0] = res
    out = np.concatenate([res.results[i]["out"].astype(np.float32)
                          for i in range(N_CORES)], axis=0)
    return out


# revision 11
# speedup vs baseline: 1.3454x; 1.2824x over previous
"""Trainium2 Bass kernel for nn_AFF_1116691497756 (dense_cnn, AFF-style fusion).

Pure data parallelism over batch (32 -> 4 per core, 8 cores). BN folded into
conv weights on host. Inputs ship as bf16 pre-halved (0.5*x, exact scaling);
output returns bf16 and is widened on host.

Key structure per core sample:
  S  = Ah+Bh, D = Ah-Bh                          [DVE TT bf16 2x]
  mm1: psum[0:64]  = 2*W1e0 @ S   (h1 pre-act)   [PE, K=256]
       psum[64:128]= (2/L)*W1e1 @ S (pooled rows, summed over L via ACT accum)
  h1 = relu(psum[0:64] + B1e0)                   [ACT from PSUM]
  pool1: relu(sum rows + B1e1) -> W2e1 -> bias1  [tiny]
  z1 = W2e0 @ h1                                 [PE K=64]
  T1 = tanh(0.5*(z1+bias1)) (= 2*wei-1)          [ACT from PSUM]
  g1 = sigmoid(-(z1+bias1)) (= 1-wei)            [ACT from PSUM]
  XQ = S + D*T1 (= xo1);  V = Ch*g1              [DVE TT]
  mm3: psum[0:64]  = W1e2@XQ + 2*W1e2@Ch  (= W1e2@(xo1+x_c))
       psum[64:128]= (1/L)*W1e3@XQ + (2/L)*W1e3@Ch (pooled rows)
  h2 = relu(... + B1e2); pool2 -> bias2; z2 = W2e2@h2
  w2s = sigmoid(z2+bias2); g2 = 0.5 + 0.5*w2s    [ACT; DVE TS 4x]
  out = XQ*g2 + V  (= 0.5*(xo1+xo2))             [DVE TT x2]
"""

import numpy as np
import ml_dtypes

import concourse.bass as bass
import concourse.bacc as bacc
import concourse.mybir as mybir
import concourse.tile as tile
from concourse.bass_utils import run_bass_kernel_spmd

EPS = 1e-5
N_CORES = 8

BF16 = mybir.dt.bfloat16
F32 = mybir.dt.float32
AOP = mybir.AluOpType
AF = mybir.ActivationFunctionType


class Cfg:
    def __init__(self, B=32, C=256, L=4096, I=64, Lc=512):
        self.B, self.C, self.L, self.I, self.Lc = B, C, L, I, Lc
        self.BL = B // N_CORES          # samples per core
        self.CH = C // 128              # C partition halves (2)
        self.NLC = L // Lc              # L chunks (8)
        assert C % 128 == 0 and L % Lc == 0 and self.NLC % 2 == 0
        assert I == 64 and self.CH == 2


def build(cfg: Cfg):
    """Build the per-core SPMD program. Returns compiled Bacc."""
    BL, CH, L, I, Lc, NLC = cfg.BL, cfg.CH, cfg.L, cfg.I, cfg.Lc, cfg.NLC
    C = cfg.C
    NG = NLC // 2                       # psum groups (2 chunks each)

    nc = bacc.Bacc("TRN2", target_bir_lowering=False, debug=False,
                   num_devices=N_CORES)

    # ---- DRAM parameters ----
    xa = nc.declare_dram_parameter("xa", [BL, C, L], BF16, isOutput=False)
    xb = nc.declare_dram_parameter("xb", [BL, C, L], BF16, isOutput=False)
    xc = nc.declare_dram_parameter("xc", [BL, C, L], BF16, isOutput=False)
    # mm1/mm3 weights: [K=128, CH, M=128] with pooled weights in cols 64:128
    lt1 = nc.declare_dram_parameter("lt1", [128, CH, 128], BF16,
                                    isOutput=False)
    lt3a = nc.declare_dram_parameter("lt3a", [128, CH, 128], BF16,
                                     isOutput=False)
    lt3b = nc.declare_dram_parameter("lt3b", [128, CH, 128], BF16,
                                     isOutput=False)
    # mm2/mm4 weights: [K=64, CH, 128]
    lt2 = nc.declare_dram_parameter("lt2", [I, CH, 128], BF16, isOutput=False)
    lt4 = nc.declare_dram_parameter("lt4", [I, CH, 128], BF16, isOutput=False)
    # pooled second-layer weights, rows 64:128 hold W2p^T (for base-64 rhs)
    ltp2 = nc.declare_dram_parameter("ltp2", [128, CH, 128], BF16,
                                     isOutput=False)
    ltp4 = nc.declare_dram_parameter("ltp4", [128, CH, 128], BF16,
                                     isOutput=False)
    br1 = nc.declare_dram_parameter("br1", [128, 1], F32, isOutput=False)
    br2 = nc.declare_dram_parameter("br2", [128, 1], F32, isOutput=False)
    bp1 = nc.declare_dram_parameter("bp1", [128, 1], F32, isOutput=False)
    bp3 = nc.declare_dram_parameter("bp3", [128, 1], F32, isOutput=False)
    BB1 = nc.declare_dram_parameter("BB1", [128, CH], F32, isOutput=False)
    BB2 = nc.declare_dram_parameter("BB2", [128, CH], F32, isOutput=False)
    out = nc.declare_dram_parameter("out", [BL, C, L], BF16, isOutput=True)

    with tile.TileContext(nc) as tc:
        with (
            tc.tile_pool(name="const", bufs=1) as cpool,
            tc.tile_pool(name="rows_ab", bufs=4) as abpool,     # Ah,Bh rows
            tc.tile_pool(name="rows_c", bufs=3) as cpool_rows,  # Ch rows
            tc.tile_pool(name="ch_s", bufs=12) as spool,        # S chunks
            tc.tile_pool(name="ch_d", bufs=12) as dpool,        # D chunks
            tc.tile_pool(name="ch_x", bufs=12) as xpool,        # XQ chunks
            tc.tile_pool(name="ch_v", bufs=12) as vpool,        # V chunks
            tc.tile_pool(name="ch_sig", bufs=6) as sigpool,     # T1/g1/w2s
            tc.tile_pool(name="ch_tmp", bufs=6) as tmppool,     # m/n/g2
            tc.tile_pool(name="ch_out", bufs=4) as outpool,
            tc.tile_pool(name="junk", bufs=1) as junkpool,
            tc.tile_pool(name="hsb", bufs=6) as hpool,          # h1/h2 sbuf
            tc.tile_pool(name="small", bufs=2 * BL) as smallpool,
            tc.tile_pool(name="hpsum", bufs=2, space="PSUM") as ph_pool,
            tc.tile_pool(name="zpsum", bufs=2, space="PSUM") as pz_pool,
        ):
            # ---- load constants to SBUF ----
            def cload(ap, shape, dtype, nm):
                t = cpool.tile(shape, dtype, name=nm, tag=nm)
                nc.sync.dma_start(t[:], ap[:])
                return t

            c_lt1 = cload(lt1, [128, CH, 128], BF16, "c_lt1")
            c_lt3a = cload(lt3a, [128, CH, 128], BF16, "c_lt3a")
            c_lt3b = cload(lt3b, [128, CH, 128], BF16, "c_lt3b")
            c_lt2 = cload(lt2, [I, CH, 128], BF16, "c_lt2")
            c_lt4 = cload(lt4, [I, CH, 128], BF16, "c_lt4")
            c_ltp2 = cload(ltp2, [128, CH, 128], BF16, "c_ltp2")
            c_ltp4 = cload(ltp4, [128, CH, 128], BF16, "c_ltp4")
            c_br1 = cload(br1, [128, 1], F32, "c_br1")
            c_br2 = cload(br2, [128, 1], F32, "c_br2")
            c_bp1 = cload(bp1, [128, 1], F32, "c_bp1")
            c_bp3 = cload(bp3, [128, 1], F32, "c_bp3")
            c_BB1 = cload(BB1, [128, CH], F32, "c_BB1")
            c_BB2 = cload(BB2, [128, CH], F32, "c_BB2")

            junk = junkpool.tile([128, 2 * Lc], BF16)

            def pooled_head(s4, cols, c_bpA, c_ltpB, nm):
                """Reduce pooled partial sums -> relu -> W2p -> att psum."""
                pre = smallpool.tile([128, 1], F32, tag="pre",
                                     name=f"pre{nm}")
                nc.vector.tensor_reduce(pre[64:128, 0:1], s4[64:128, cols],
                                        mybir.AxisListType.X, AOP.add)
                hp = smallpool.tile([128, 1], BF16, tag="hp", name=f"hp{nm}")
                nc.scalar.activation(hp[64:128, 0:1], pre[64:128, 0:1],
                                     AF.Relu, bias=c_bpA[64:128, 0:1],
                                     scale=1.0)
                pat = pz_pool.tile([128, CH], F32, tag="pz", name=f"pat{nm}")
                for mh in range(CH):
                    nc.tensor.matmul(pat[:, mh:mh + 1],
                                     c_ltpB[64:128, mh, :],
                                     hp[64:128, 0:1], start=True, stop=True)
                return pat

            for b in range(BL):
                # -------- phase A: load + S/D ---------------------------
                s4 = smallpool.tile([128, 2 * NG], F32, tag="s4",
                                    name=f"s4_{b}")
                tA, tB, tC, tS, tD = [], [], [], [], []
                for kh in range(CH):
                    ta = abpool.tile([128, L], BF16, tag="ab",
                                     name=f"ta_{b}_{kh}")
                    nc.sync.dma_start(ta[:], xa[b, kh * 128:(kh + 1) * 128, :])
                    tA.append(ta)
                    tb = abpool.tile([128, L], BF16, tag="ab",
                                     name=f"tb_{b}_{kh}")
                    nc.sync.dma_start(tb[:], xb[b, kh * 128:(kh + 1) * 128, :])
                    tB.append(tb)
                    tcc = cpool_rows.tile([128, L], BF16, tag="c",
                                          name=f"tc_{b}_{kh}")
                    nc.sync.dma_start(tcc[:],
                                      xc[b, kh * 128:(kh + 1) * 128, :])
                    tC.append(tcc)
                    s_chunks, d_chunks = [], []
                    for g in range(NG):
                        gsl = slice(2 * g * Lc, 2 * (g + 1) * Lc)
                        sc = spool.tile([128, 2 * Lc], BF16, tag="s",
                                        name=f"ts_{b}_{kh}_{g}")
                        nc.vector.tensor_tensor(sc[:], ta[:, gsl],
                                                tb[:, gsl], AOP.add)
                        s_chunks.append(sc)
                        dc = dpool.tile([128, 2 * Lc], BF16, tag="d",
                                        name=f"td_{b}_{kh}_{g}")
                        nc.vector.tensor_tensor(dc[:], ta[:, gsl],
                                                tb[:, gsl], AOP.subtract)
                        d_chunks.append(dc)
                    tS.append(s_chunks)
                    tD.append(d_chunks)

                # -------- phase B1: mm1 (+pooled rows) + relu ----------
                h1s = []
                for g in range(NG):
                    ph = ph_pool.tile([128, 2 * Lc], F32, tag="ph",
                                      name=f"ph_{b}_{g}")
                    for q in range(2):
                        for kh in range(CH):
                            nc.tensor.matmul(
                                ph[:, q * Lc:(q + 1) * Lc],
                                c_lt1[:, kh, :],
                                tS[kh][g][:, q * Lc:(q + 1) * Lc],
                                start=(kh == 0), stop=(kh == CH - 1))
                    h1 = hpool.tile([I, 2 * Lc], BF16, tag="h",
                                    name=f"h1_{b}_{g}")
                    nc.scalar.activation(h1[:], ph[0:I, :], AF.Relu,
                                         bias=c_br1[0:I, 0:1], scale=1.0)
                    h1s.append(h1)
                    nc.scalar.activation(
                        junk[64:128, :], ph[64:128, :], AF.Copy, bias=0.0,
                        scale=1.0, accum_out=s4[64:128, g:g + 1])

                # -------- pooled branch 1 ------------------------------
                pat1 = pooled_head(s4, slice(0, NG), c_bp1, c_ltp2, f"1_{b}")
                bias1h = smallpool.tile([128, CH], F32, tag="bias1h",
                                        name=f"bias1h_{b}")
                nb1 = smallpool.tile([128, CH], F32, tag="nb1",
                                     name=f"nb1_{b}")
                for mh in range(CH):
                    nc.vector.tensor_scalar(
                        bias1h[:, mh:mh + 1], pat1[:, mh:mh + 1],
                        c_BB1[:, mh:mh + 1], 0.5, AOP.add, AOP.mult)
                    nc.vector.tensor_scalar(
                        nb1[:, mh:mh + 1], pat1[:, mh:mh + 1],
                        c_BB1[:, mh:mh + 1], -1.0, AOP.add, AOP.mult)

                # -------- phase B2: mm2 -> T1/g1 -> XQ/V (chunked) -----
                tX = [[None] * NG for _ in range(CH)]
                tV = [[None] * NG for _ in range(CH)]
                for g in range(NG):
                    gsl = slice(2 * g * Lc, 2 * (g + 1) * Lc)
                    for mh in range(CH):
                        pz = pz_pool.tile([128, 2 * Lc], F32, tag="pz",
                                          name=f"pz_{b}_{g}_{mh}")
                        for q in range(2):
                            nc.tensor.matmul(
                                pz[:, q * Lc:(q + 1) * Lc],
                                c_lt2[:, mh, :],
                                h1s[g][:, q * Lc:(q + 1) * Lc],
                                start=True, stop=True)
                        t1c = sigpool.tile([128, 2 * Lc], BF16, tag="sig",
                                           name=f"T1_{b}_{g}_{mh}")
                        nc.scalar.activation(
                            t1c[:], pz[:], AF.Tanh,
                            bias=bias1h[:, mh:mh + 1], scale=0.5)
                        g1c = sigpool.tile([128, 2 * Lc], BF16, tag="sig",
                                           name=f"g1_{b}_{g}_{mh}")
                        nc.scalar.activation(
                            g1c[:], pz[:], AF.Sigmoid,
                            bias=nb1[:, mh:mh + 1], scale=-1.0)
                        m_t = tmppool.tile([128, 2 * Lc], BF16, tag="m",
                                           name=f"m_{b}_{g}_{mh}")
                        nc.vector.tensor_tensor(m_t[:], tD[mh][g][:],
                                                t1c[:], AOP.mult)
                        x_t = xpool.tile([128, 2 * Lc], BF16, tag="x",
                                         name=f"x_{b}_{g}_{mh}")
                        nc.vector.tensor_tensor(x_t[:], tS[mh][g][:],
                                                m_t[:], AOP.add)
                        tX[mh][g] = x_t
                        v_t = vpool.tile([128, 2 * Lc], BF16, tag="v",
                                         name=f"v_{b}_{g}_{mh}")
                        nc.vector.tensor_tensor(v_t[:], tC[mh][:, gsl],
                                                g1c[:], AOP.mult)
                        tV[mh][g] = v_t

                # -------- phase C1: mm3 (+pooled rows) + relu ----------
                h2s = []
                for g in range(NG):
                    ph2 = ph_pool.tile([128, 2 * Lc], F32, tag="ph",
                                       name=f"ph2_{b}_{g}")
                    for q in range(2):
                        lc = 2 * g + q
                        sl = slice(lc * Lc, (lc + 1) * Lc)
                        qsl = slice(q * Lc, (q + 1) * Lc)
                        i_mm = 0
                        for kh in range(CH):
                            nc.tensor.matmul(
                                ph2[:, qsl], c_lt3a[:, kh, :],
                                tX[kh][g][:, qsl],
                                start=(i_mm == 0), stop=False)
                            i_mm += 1
                        for kh in range(CH):
                            i_mm += 1
                            nc.tensor.matmul(
                                ph2[:, qsl], c_lt3b[:, kh, :],
                                tC[kh][:, sl],
                                start=False, stop=(i_mm == 2 * CH))
                    h2 = hpool.tile([I, 2 * Lc], BF16, tag="h",
                                    name=f"h2_{b}_{g}")
                    nc.scalar.activation(h2[:], ph2[0:I, :], AF.Relu,
                                         bias=c_br2[0:I, 0:1], scale=1.0)
                    h2s.append(h2)
                    nc.scalar.activation(
                        junk[64:128, :], ph2[64:128, :], AF.Copy, bias=0.0,
                        scale=1.0, accum_out=s4[64:128, NG + g:NG + g + 1])

                # -------- pooled branch 2 ------------------------------
                pat2 = pooled_head(s4, slice(NG, 2 * NG), c_bp3, c_ltp4,
                                   f"2_{b}")
                bias2 = smallpool.tile([128, CH], F32, tag="bias2",
                                       name=f"bias2_{b}")
                for mh in range(CH):
                    nc.vector.tensor_scalar(
                        bias2[:, mh:mh + 1], pat2[:, mh:mh + 1],
                        c_BB2[:, mh:mh + 1], None, AOP.add)

                # -------- phase C2: mm4 -> w2s -> out (chunked) --------
                for g in range(NG):
                    gsl = slice(2 * g * Lc, 2 * (g + 1) * Lc)
                    for mh in range(CH):
                        pz2 = pz_pool.tile([128, 2 * Lc], F32, tag="pz",
                                           name=f"pz2_{b}_{g}_{mh}")
                        for q in range(2):
                            nc.tensor.matmul(
                                pz2[:, q * Lc:(q + 1) * Lc],
                                c_lt4[:, mh, :],
                                h2s[g][:, q * Lc:(q + 1) * Lc],
                                start=True, stop=True)
                        w2c = sigpool.tile([128, 2 * Lc], BF16, tag="sig",
                                           name=f"w2s_{b}_{g}_{mh}")
                        nc.scalar.activation(
                            w2c[:], pz2[:], AF.Sigmoid,
                            bias=bias2[:, mh:mh + 1], scale=1.0)
                        g2 = tmppool.tile([128, 2 * Lc], BF16, tag="m",
                                          name=f"g2_{b}_{g}_{mh}")
                        nc.vector.tensor_scalar(g2[:], w2c[:], 0.5, 0.5,
                                                AOP.mult, AOP.add)
                        n_t = tmppool.tile([128, 2 * Lc], BF16, tag="m",
                                           name=f"n_{b}_{g}_{mh}")
                        nc.vector.tensor_tensor(n_t[:], tX[mh][g][:], g2[:],
                                                AOP.mult)
                        ob = outpool.tile([128, 2 * Lc], BF16, tag="ob",
                                          name=f"ob_{b}_{g}_{mh}")
                        nc.vector.tensor_tensor(ob[:], n_t[:], tV[mh][g][:],
                                                AOP.add)
                        nc.sync.dma_start(
                            out[b, mh * 128:(mh + 1) * 128, gsl], ob[:])

    nc.compile()
    return nc


def host_params(w1, b1, bn1_g, bn1_b, bn1_m, bn1_v,
                w2, b2, bn2_g, bn2_b, bn2_m, bn2_v, cfg: Cfg):
    """Fold BN into conv weights; build device param arrays."""
    CH, I, L = cfg.CH, cfg.I, cfg.L
    w1 = w1.astype(np.float64); w2 = w2.astype(np.float64)
    s1 = bn1_g / np.sqrt(bn1_v + EPS)            # [4, I]
    t1 = bn1_b - bn1_m * s1
    W1e = s1[:, :, None] * w1                    # [4, I, C]
    B1e = s1 * b1 + t1                           # [4, I]
    s2 = bn2_g / np.sqrt(bn2_v + EPS)            # [4, C]
    t2 = bn2_b - bn2_m * s2
    W2e = s2[:, :, None] * w2                    # [4, C, I]
    B2e = s2 * b2 + t2                           # [4, C]

    def to_bf(x):
        return np.ascontiguousarray(x.astype(ml_dtypes.bfloat16))

    def kxm_ext(Wf, sf, Wp, sp):
        # [I,C] full (scale sf) + [I,C] pooled (scale sp)
        # -> lhsT [128, CH, 128]: cols 0:64 full, 64:128 pooled
        full = (Wf.T * sf).reshape(CH, 128, I)       # [CH, 128, I]
        pool = (Wp.T * sp).reshape(CH, 128, I)
        t = np.concatenate([full, pool], axis=2)      # [CH, 128, 128]
        return to_bf(t.transpose(1, 0, 2))            # [128, CH, 128]

    def mt(W):  # W [C, I] -> lhsT [I, CH, 128]
        return to_bf(W.T.reshape(I, CH, 128))

    def dup_mt(W):  # W [C, I] -> [128, CH, 128], rows 64:128 = W^T slices
        t = W.T.reshape(I, CH, 128)
        return to_bf(np.concatenate([t, t], axis=0))

    p = {
        "lt1": kxm_ext(W1e[0], 2.0, W1e[1], 2.0 / L),
        "lt3a": kxm_ext(W1e[2], 1.0, W1e[3], 1.0 / L),
        "lt3b": kxm_ext(W1e[2], 2.0, W1e[3], 2.0 / L),
        "lt2": mt(W2e[0]),
        "lt4": mt(W2e[2]),
        "ltp2": dup_mt(W2e[1]),
        "ltp4": dup_mt(W2e[3]),
        "br1": np.concatenate([B1e[0], B1e[0]]).astype(np.float32)
                 .reshape(128, 1),
        "br2": np.concatenate([B1e[2], B1e[2]]).astype(np.float32)
                 .reshape(128, 1),
        "bp1": np.concatenate([B1e[1], B1e[1]]).astype(np.float32)
                 .reshape(128, 1),
        "bp3": np.concatenate([B1e[3], B1e[3]]).astype(np.float32)
                 .reshape(128, 1),
        "BB1": (B2e[0] + B2e[1]).astype(np.float32).reshape(CH, 128).T.copy(),
        "BB2": (B2e[2] + B2e[3]).astype(np.float32).reshape(CH, 128).T.copy(),
    }
    return p


_CACHE = {}


def _get_nc(cfg: Cfg):
    key = (cfg.B, cfg.C, cfg.L, cfg.I, cfg.Lc)
    if key not in _CACHE:
        _CACHE[key] = build(cfg)
    return _CACHE[key]


LAST_RESULT = [None]


def kernel(x_a, x_b, x_c, w1, b1, bn1_g, bn1_b, bn1_m, bn1_v,
           w2, b2, bn2_g, bn2_b, bn2_m, bn2_v):
    cfg = Cfg(B=x_a.shape[0], C=x_a.shape[1], L=x_a.shape[2], I=w1.shape[1])
    nc = _get_nc(cfg)
    params = host_params(np.asarray(w1), np.asarray(b1), np.asarray(bn1_g),
                         np.asarray(bn1_b), np.asarray(bn1_m),
                         np.asarray(bn1_v), np.asarray(w2), np.asarray(b2),
                         np.asarray(bn2_g), np.asarray(bn2_b),
                         np.asarray(bn2_m), np.asarray(bn2_v), cfg)
    BL = cfg.BL
    bf = ml_dtypes.bfloat16
    in_maps = []
    for i in range(N_CORES):
        sl = slice(i * BL, (i + 1) * BL)
        m = dict(params)
        m["xa"] = np.ascontiguousarray((np.asarray(x_a[sl]) * 0.5).astype(bf))
        m["xb"] = np.ascontiguousarray((np.asarray(x_b[sl]) * 0.5).astype(bf))
        m["xc"] = np.ascontiguousarray((np.asarray(x_c[sl]) * 0.5).astype(bf))
        in_maps.append(m)

    import os
    res = run_bass_kernel_spmd(nc, in_maps, core_ids=list(range(N_CORES)),
                               trace=bool(os.environ.get("BASS_TRACE")))
    LAST_RESULT[0] = res
    out = np.concatenate([res.results[i]["out"].astype(np.float32)
                          for i in range(N_CORES)], axis=0)
    return out


# revision 13
# speedup vs baseline: 1.4873x; 1.1055x over previous
"""Trainium2 Bass kernel for nn_AFF_1116691497756 (dense_cnn, AFF-style fusion).

Pure data parallelism over batch (32 -> 4 per core, 8 cores). BN folded into
conv weights on host. Inputs ship as bf16 pre-halved (0.5*x, exact scaling);
output returns bf16 and is widened on host.

Key structure per core sample:
  S  = Ah+Bh, D = Ah-Bh                          [DVE TT bf16 2x]
  mm1: psum[0:64]  = 2*W1e0 @ S   (h1 pre-act)   [PE, K=256]
       psum[64:128]= (2/L)*W1e1 @ S (pooled rows, summed over L via ACT accum)
  h1 = relu(psum[0:64] + B1e0)                   [ACT from PSUM]
  pool1: relu(sum rows + B1e1) -> W2e1 -> bias1  [tiny]
  z1 = W2e0 @ h1                                 [PE K=64]
  T1 = tanh(0.5*(z1+bias1)) (= 2*wei-1)          [ACT from PSUM]
  g1 = sigmoid(-(z1+bias1)) (= 1-wei)            [ACT from PSUM]
  XQ = S + D*T1 (= xo1);  V = Ch*g1              [DVE TT]
  mm3: psum[0:64]  = W1e2@XQ + 2*W1e2@Ch  (= W1e2@(xo1+x_c))
       psum[64:128]= (1/L)*W1e3@XQ + (2/L)*W1e3@Ch (pooled rows)
  h2 = relu(... + B1e2); pool2 -> bias2; z2 = W2e2@h2
  w2s = sigmoid(z2+bias2); g2 = 0.5 + 0.5*w2s    [ACT; DVE TS 4x]
  out = XQ*g2 + V  (= 0.5*(xo1+xo2))             [DVE TT x2]
"""

import numpy as np
import ml_dtypes

import concourse.bass as bass
import concourse.bacc as bacc
import concourse.mybir as mybir
import concourse.tile as tile
from concourse.bass_utils import run_bass_kernel_spmd

EPS = 1e-5
N_CORES = 8

BF16 = mybir.dt.bfloat16
F32 = mybir.dt.float32
AOP = mybir.AluOpType
AF = mybir.ActivationFunctionType


class Cfg:
    def __init__(self, B=32, C=256, L=4096, I=64, Lc=512):
        self.B, self.C, self.L, self.I, self.Lc = B, C, L, I, Lc
        self.BL = B // N_CORES          # samples per core
        self.CH = C // 128              # C partition halves (2)
        self.NLC = L // Lc              # L chunks (8)
        assert C % 128 == 0 and L % Lc == 0 and self.NLC % 2 == 0
        assert I == 64 and self.CH == 2


def build(cfg: Cfg):
    """Build the per-core SPMD program. Returns compiled Bacc."""
    BL, CH, L, I, Lc, NLC = cfg.BL, cfg.CH, cfg.L, cfg.I, cfg.Lc, cfg.NLC
    C = cfg.C
    NG = NLC // 2                       # psum groups (2 chunks each)

    nc = bacc.Bacc("TRN2", target_bir_lowering=False, debug=False,
                   num_devices=N_CORES)

    # ---- DRAM parameters ----
    xa = nc.declare_dram_parameter("xa", [BL, C, L], BF16, isOutput=False)
    xb = nc.declare_dram_parameter("xb", [BL, C, L], BF16, isOutput=False)
    xc = nc.declare_dram_parameter("xc", [BL, C, L], BF16, isOutput=False)
    # mm1/mm3 weights: [K=128, CH, M=128] with pooled weights in cols 64:128
    lt1 = nc.declare_dram_parameter("lt1", [128, CH, 128], BF16,
                                    isOutput=False)
    lt3a = nc.declare_dram_parameter("lt3a", [128, CH, 128], BF16,
                                     isOutput=False)
    lt3b = nc.declare_dram_parameter("lt3b", [128, CH, 128], BF16,
                                     isOutput=False)
    # mm2/mm4 weights: [K=64, CH, 128]
    lt2 = nc.declare_dram_parameter("lt2", [I, CH, 128], BF16, isOutput=False)
    lt4 = nc.declare_dram_parameter("lt4", [I, CH, 128], BF16, isOutput=False)
    # pooled second-layer weights, rows 64:128 hold W2p^T (for base-64 rhs)
    ltp2 = nc.declare_dram_parameter("ltp2", [128, CH, 128], BF16,
                                     isOutput=False)
    ltp4 = nc.declare_dram_parameter("ltp4", [128, CH, 128], BF16,
                                     isOutput=False)
    br1 = nc.declare_dram_parameter("br1", [128, 1], F32, isOutput=False)
    br2 = nc.declare_dram_parameter("br2", [128, 1], F32, isOutput=False)
    bp1 = nc.declare_dram_parameter("bp1", [128, 1], F32, isOutput=False)
    bp3 = nc.declare_dram_parameter("bp3", [128, 1], F32, isOutput=False)
    BB1 = nc.declare_dram_parameter("BB1", [128, CH], F32, isOutput=False)
    BB2 = nc.declare_dram_parameter("BB2", [128, CH], F32, isOutput=False)
    out = nc.declare_dram_parameter("out", [BL, C, L], BF16, isOutput=True)

    with tile.TileContext(nc) as tc:
        with (
            tc.tile_pool(name="const", bufs=1) as cpool,
            tc.tile_pool(name="rows_ab", bufs=4) as abpool,     # Ah,Bh rows
            tc.tile_pool(name="rows_c", bufs=3) as cpool_rows,  # Ch rows
            tc.tile_pool(name="ch_s", bufs=12) as spool,        # S chunks
            tc.tile_pool(name="ch_d", bufs=12) as dpool,        # D chunks
            tc.tile_pool(name="ch_x", bufs=12) as xpool,        # XQ chunks
            tc.tile_pool(name="ch_v", bufs=12) as vpool,        # V chunks
            tc.tile_pool(name="ch_sig", bufs=6) as sigpool,     # T1/g1/w2s
            tc.tile_pool(name="ch_tmp", bufs=6) as tmppool,     # m/n/g2
            tc.tile_pool(name="ch_out", bufs=4) as outpool,
            tc.tile_pool(name="junk", bufs=1) as junkpool,
            tc.tile_pool(name="hsb", bufs=6) as hpool,          # h1/h2 sbuf
            tc.tile_pool(name="small", bufs=2 * BL) as smallpool,
            tc.tile_pool(name="hpsum", bufs=2, space="PSUM") as ph_pool,
            tc.tile_pool(name="zpsum", bufs=2, space="PSUM") as pz_pool,
        ):
            # ---- load constants to SBUF ----
            def cload(ap, shape, dtype, nm):
                t = cpool.tile(shape, dtype, name=nm, tag=nm)
                nc.sync.dma_start(t[:], ap[:])
                return t

            c_lt1 = cload(lt1, [128, CH, 128], BF16, "c_lt1")
            c_lt3a = cload(lt3a, [128, CH, 128], BF16, "c_lt3a")
            c_lt3b = cload(lt3b, [128, CH, 128], BF16, "c_lt3b")
            c_lt2 = cload(lt2, [I, CH, 128], BF16, "c_lt2")
            c_lt4 = cload(lt4, [I, CH, 128], BF16, "c_lt4")
            c_ltp2 = cload(ltp2, [128, CH, 128], BF16, "c_ltp2")
            c_ltp4 = cload(ltp4, [128, CH, 128], BF16, "c_ltp4")
            c_br1 = cload(br1, [128, 1], F32, "c_br1")
            c_br2 = cload(br2, [128, 1], F32, "c_br2")
            c_bp1 = cload(bp1, [128, 1], F32, "c_bp1")
            c_bp3 = cload(bp3, [128, 1], F32, "c_bp3")
            c_BB1 = cload(BB1, [128, CH], F32, "c_BB1")
            c_BB2 = cload(BB2, [128, CH], F32, "c_BB2")

            junk = junkpool.tile([128, 2 * Lc], BF16)

            def pooled_head(s4, cols, c_bpA, c_ltpB, nm):
                """Reduce pooled partial sums -> relu -> W2p -> att psum."""
                pre = smallpool.tile([128, 1], F32, tag="pre",
                                     name=f"pre{nm}")
                nc.vector.tensor_reduce(pre[64:128, 0:1], s4[64:128, cols],
                                        mybir.AxisListType.X, AOP.add)
                hp = smallpool.tile([128, 1], BF16, tag="hp", name=f"hp{nm}")
                nc.scalar.activation(hp[64:128, 0:1], pre[64:128, 0:1],
                                     AF.Relu, bias=c_bpA[64:128, 0:1],
                                     scale=1.0)
                pat = pz_pool.tile([128, CH], F32, tag="pz", name=f"pat{nm}")
                for mh in range(CH):
                    nc.tensor.matmul(pat[:, mh:mh + 1],
                                     c_ltpB[64:128, mh, :],
                                     hp[64:128, 0:1], start=True, stop=True)
                return pat

            for b in range(BL):
                # -------- phase A: load + S/D ---------------------------
                s4 = smallpool.tile([128, 2 * NG], F32, tag="s4",
                                    name=f"s4_{b}")
                tA, tB, tC, tS, tD = [], [], [], [], []
                for kh in range(CH):
                    ta = abpool.tile([128, L], BF16, tag="ab",
                                     name=f"ta_{b}_{kh}")
                    nc.sync.dma_start(ta[:], xa[b, kh * 128:(kh + 1) * 128, :])
                    tA.append(ta)
                    tb = abpool.tile([128, L], BF16, tag="ab",
                                     name=f"tb_{b}_{kh}")
                    nc.sync.dma_start(tb[:], xb[b, kh * 128:(kh + 1) * 128, :])
                    tB.append(tb)
                    tcc = cpool_rows.tile([128, L], BF16, tag="c",
                                          name=f"tc_{b}_{kh}")
                    nc.sync.dma_start(tcc[:],
                                      xc[b, kh * 128:(kh + 1) * 128, :])
                    tC.append(tcc)
                    s_chunks, d_chunks = [], []
                    for g in range(NG):
                        gsl = slice(2 * g * Lc, 2 * (g + 1) * Lc)
                        sc = spool.tile([128, 2 * Lc], BF16, tag="s",
                                        name=f"ts_{b}_{kh}_{g}")
                        nc.vector.tensor_tensor(sc[:], ta[:, gsl],
                                                tb[:, gsl], AOP.add)
                        s_chunks.append(sc)
                        dc = dpool.tile([128, 2 * Lc], BF16, tag="d",
                                        name=f"td_{b}_{kh}_{g}")
                        nc.vector.tensor_tensor(dc[:], ta[:, gsl],
                                                tb[:, gsl], AOP.subtract)
                        d_chunks.append(dc)
                    tS.append(s_chunks)
                    tD.append(d_chunks)

                # -------- phase B1: mm1 (+pooled rows) + relu ----------
                h1s = []
                for g in range(NG):
                    ph = ph_pool.tile([128, 2 * Lc], F32, tag="ph",
                                      name=f"ph_{b}_{g}")
                    for q in range(2):
                        for kh in range(CH):
                            nc.tensor.matmul(
                                ph[:, q * Lc:(q + 1) * Lc],
                                c_lt1[:, kh, :],
                                tS[kh][g][:, q * Lc:(q + 1) * Lc],
                                start=(kh == 0), stop=(kh == CH - 1))
                    h1 = hpool.tile([I, 2 * Lc], BF16, tag="h",
                                    name=f"h1_{b}_{g}")
                    nc.scalar.activation(h1[:], ph[0:I, :], AF.Relu,
                                         bias=c_br1[0:I, 0:1], scale=1.0)
                    h1s.append(h1)
                    nc.scalar.activation(
                        junk[64:128, :], ph[64:128, :], AF.Copy, bias=0.0,
                        scale=1.0, accum_out=s4[64:128, g:g + 1])

                # -------- pooled branch 1 ------------------------------
                pat1 = pooled_head(s4, slice(0, NG), c_bp1, c_ltp2, f"1_{b}")
                bias1h = smallpool.tile([128, CH], F32, tag="bias1h",
                                        name=f"bias1h_{b}")
                for mh in range(CH):
                    nc.vector.tensor_scalar(
                        bias1h[:, mh:mh + 1], pat1[:, mh:mh + 1],
                        c_BB1[:, mh:mh + 1], 0.5, AOP.add, AOP.mult)

                # -------- phase B2: mm2 -> T1/g1 -> XQ/V (chunked) -----
                tX = [[None] * NG for _ in range(CH)]
                tV = [[None] * NG for _ in range(CH)]
                for g in range(NG):
                    gsl = slice(2 * g * Lc, 2 * (g + 1) * Lc)
                    for mh in range(CH):
                        pz = pz_pool.tile([128, 2 * Lc], F32, tag="pz",
                                          name=f"pz_{b}_{g}_{mh}")
                        for q in range(2):
                            nc.tensor.matmul(
                                pz[:, q * Lc:(q + 1) * Lc],
                                c_lt2[:, mh, :],
                                h1s[g][:, q * Lc:(q + 1) * Lc],
                                start=True, stop=True)
                        t1c = sigpool.tile([128, 2 * Lc], BF16, tag="sig",
                                           name=f"T1_{b}_{g}_{mh}")
                        nc.scalar.activation(
                            t1c[:], pz[:], AF.Tanh,
                            bias=bias1h[:, mh:mh + 1], scale=0.5)
                        g1c = sigpool.tile([128, 2 * Lc], BF16, tag="sig",
                                           name=f"g1_{b}_{g}_{mh}")
                        nc.vector.tensor_scalar(g1c[:], t1c[:], -0.5, 0.5,
                                                AOP.mult, AOP.add)
                        m_t = tmppool.tile([128, 2 * Lc], BF16, tag="m",
                                           name=f"m_{b}_{g}_{mh}")
                        nc.vector.tensor_tensor(m_t[:], tD[mh][g][:],
                                                t1c[:], AOP.mult)
                        x_t = xpool.tile([128, 2 * Lc], BF16, tag="x",
                                         name=f"x_{b}_{g}_{mh}")
                        nc.vector.tensor_tensor(x_t[:], tS[mh][g][:],
                                                m_t[:], AOP.add)
                        tX[mh][g] = x_t
                        v_t = vpool.tile([128, 2 * Lc], BF16, tag="v",
                                         name=f"v_{b}_{g}_{mh}")
                        nc.vector.tensor_tensor(v_t[:], tC[mh][:, gsl],
                                                g1c[:], AOP.mult)
                        tV[mh][g] = v_t

                # -------- phase C1: mm3 (+pooled rows) + relu ----------
                h2s = []
                for g in range(NG):
                    ph2 = ph_pool.tile([128, 2 * Lc], F32, tag="ph",
                                       name=f"ph2_{b}_{g}")
                    for q in range(2):
                        lc = 2 * g + q
                        sl = slice(lc * Lc, (lc + 1) * Lc)
                        qsl = slice(q * Lc, (q + 1) * Lc)
                        i_mm = 0
                        for kh in range(CH):
                            nc.tensor.matmul(
                                ph2[:, qsl], c_lt3a[:, kh, :],
                                tX[kh][g][:, qsl],
                                start=(i_mm == 0), stop=False)
                            i_mm += 1
                        for kh in range(CH):
                            i_mm += 1
                            nc.tensor.matmul(
                                ph2[:, qsl], c_lt3b[:, kh, :],
                                tC[kh][:, sl],
                                start=False, stop=(i_mm == 2 * CH))
                    h2 = hpool.tile([I, 2 * Lc], BF16, tag="h",
                                    name=f"h2_{b}_{g}")
                    nc.scalar.activation(h2[:], ph2[0:I, :], AF.Relu,
                                         bias=c_br2[0:I, 0:1], scale=1.0)
                    h2s.append(h2)
                    nc.scalar.activation(
                        junk[64:128, :], ph2[64:128, :], AF.Copy, bias=0.0,
                        scale=1.0, accum_out=s4[64:128, NG + g:NG + g + 1])

                # -------- pooled branch 2 ------------------------------
                pat2 = pooled_head(s4, slice(NG, 2 * NG), c_bp3, c_ltp4,
                                   f"2_{b}")
                bias2 = smallpool.tile([128, CH], F32, tag="bias2",
                                       name=f"bias2_{b}")
                for mh in range(CH):
                    nc.vector.tensor_scalar(
                        bias2[:, mh:mh + 1], pat2[:, mh:mh + 1],
                        c_BB2[:, mh:mh + 1], None, AOP.add)

                # -------- phase C2: mm4 -> w2s -> out (chunked) --------
                for g in range(NG):
                    gsl = slice(2 * g * Lc, 2 * (g + 1) * Lc)
                    for mh in range(CH):
                        pz2 = pz_pool.tile([128, 2 * Lc], F32, tag="pz",
                                           name=f"pz2_{b}_{g}_{mh}")
                        for q in range(2):
                            nc.tensor.matmul(
                                pz2[:, q * Lc:(q + 1) * Lc],
                                c_lt4[:, mh, :],
                                h2s[g][:, q * Lc:(q + 1) * Lc],
                                start=True, stop=True)
                        w2c = sigpool.tile([128, 2 * Lc], BF16, tag="sig",
                                           name=f"w2s_{b}_{g}_{mh}")
                        nc.scalar.activation(
                            w2c[:], pz2[:], AF.Sigmoid,
                            bias=bias2[:, mh:mh + 1], scale=1.0)
                        g2 = tmppool.tile([128, 2 * Lc], BF16, tag="m",
                                          name=f"g2_{b}_{g}_{mh}")
                        nc.vector.tensor_scalar(g2[:], w2c[:], 0.5, 0.5,
                                                AOP.mult, AOP.add)
                        n_t = tmppool.tile([128, 2 * Lc], BF16, tag="m",
                                           name=f"n_{b}_{g}_{mh}")
                        nc.vector.tensor_tensor(n_t[:], tX[mh][g][:], g2[:],
                                                AOP.mult)
                        ob = outpool.tile([128, 2 * Lc], BF16, tag="ob",
                                          name=f"ob_{b}_{g}_{mh}")
                        nc.vector.tensor_tensor(ob[:], n_t[:], tV[mh][g][:],
                                                AOP.add)
                        nc.sync.dma_start(
                            out[b, mh * 128:(mh + 1) * 128, gsl], ob[:])

    nc.compile()
    return nc


def host_params(w1, b1, bn1_g, bn1_b, bn1_m, bn1_v,
                w2, b2, bn2_g, bn2_b, bn2_m, bn2_v, cfg: Cfg):
    """Fold BN into conv weights; build device param arrays."""
    CH, I, L = cfg.CH, cfg.I, cfg.L
    w1 = w1.astype(np.float64); w2 = w2.astype(np.float64)
    s1 = bn1_g / np.sqrt(bn1_v + EPS)            # [4, I]
    t1 = bn1_b - bn1_m * s1
    W1e = s1[:, :, None] * w1                    # [4, I, C]
    B1e = s1 * b1 + t1                           # [4, I]
    s2 = bn2_g / np.sqrt(bn2_v + EPS)            # [4, C]
    t2 = bn2_b - bn2_m * s2
    W2e = s2[:, :, None] * w2                    # [4, C, I]
    B2e = s2 * b2 + t2                           # [4, C]

    def to_bf(x):
        return np.ascontiguousarray(x.astype(ml_dtypes.bfloat16))

    def kxm_ext(Wf, sf, Wp, sp):
        # [I,C] full (scale sf) + [I,C] pooled (scale sp)
        # -> lhsT [128, CH, 128]: cols 0:64 full, 64:128 pooled
        full = (Wf.T * sf).reshape(CH, 128, I)       # [CH, 128, I]
        pool = (Wp.T * sp).reshape(CH, 128, I)
        t = np.concatenate([full, pool], axis=2)      # [CH, 128, 128]
        return to_bf(t.transpose(1, 0, 2))            # [128, CH, 128]

    def mt(W):  # W [C, I] -> lhsT [I, CH, 128]
        return to_bf(W.T.reshape(I, CH, 128))

    def dup_mt(W):  # W [C, I] -> [128, CH, 128], rows 64:128 = W^T slices
        t = W.T.reshape(I, CH, 128)
        return to_bf(np.concatenate([t, t], axis=0))

    p = {
        "lt1": kxm_ext(W1e[0], 2.0, W1e[1], 2.0 / L),
        "lt3a": kxm_ext(W1e[2], 1.0, W1e[3], 1.0 / L),
        "lt3b": kxm_ext(W1e[2], 2.0, W1e[3], 2.0 / L),
        "lt2": mt(W2e[0]),
        "lt4": mt(W2e[2]),
        "ltp2": dup_mt(W2e[1]),
        "ltp4": dup_mt(W2e[3]),
        "br1": np.concatenate([B1e[0], B1e[0]]).astype(np.float32)
                 .reshape(128, 1),
        "br2": np.concatenate([B1e[2], B1e[2]]).astype(np.float32)
                 .reshape(128, 1),
        "bp1": np.concatenate([B1e[1], B1e[1]]).astype(np.float32)
                 .reshape(128, 1),
        "bp3": np.concatenate([B1e[3], B1e[3]]).astype(np.float32)
                 .reshape(128, 1),
        "BB1": (B2e[0] + B2e[1]).astype(np.float32).reshape(CH, 128).T.copy(),
        "BB2": (B2e[2] + B2e[3]).astype(np.float32).reshape(CH, 128).T.copy(),
    }
    return p


_CACHE = {}


def _get_nc(cfg: Cfg):
    key = (cfg.B, cfg.C, cfg.L, cfg.I, cfg.Lc)
    if key not in _CACHE:
        _CACHE[key] = build(cfg)
    return _CACHE[key]


LAST_RESULT = [None]


def kernel(x_a, x_b, x_c, w1, b1, bn1_g, bn1_b, bn1_m, bn1_v,
           w2, b2, bn2_g, bn2_b, bn2_m, bn2_v):
    cfg = Cfg(B=x_a.shape[0], C=x_a.shape[1], L=x_a.shape[2], I=w1.shape[1])
    nc = _get_nc(cfg)
    params = host_params(np.asarray(w1), np.asarray(b1), np.asarray(bn1_g),
                         np.asarray(bn1_b), np.asarray(bn1_m),
                         np.asarray(bn1_v), np.asarray(w2), np.asarray(b2),
                         np.asarray(bn2_g), np.asarray(bn2_b),
                         np.asarray(bn2_m), np.asarray(bn2_v), cfg)
    BL = cfg.BL
    bf = ml_dtypes.bfloat16
    in_maps = []
    for i in range(N_CORES):
        sl = slice(i * BL, (i + 1) * BL)
        m = dict(params)
        m["xa"] = np.ascontiguousarray((np.asarray(x_a[sl]) * 0.5).astype(bf))
        m["xb"] = np.ascontiguousarray((np.asarray(x_b[sl]) * 0.5).astype(bf))
        m["xc"] = np.ascontiguousarray((np.asarray(x_c[sl]) * 0.5).astype(bf))
        in_maps.append(m)

    import os
    res = run_bass_kernel_spmd(nc, in_maps, core_ids=list(range(N_CORES)),
                               trace=bool(os.environ.get("BASS_TRACE")))
    LAST_RESULT[0] = res
    out = np.concatenate([res.results[i]["out"].astype(np.float32)
                          for i in range(N_CORES)], axis=0)
    return out
